# revision 1
# baseline (speedup 1.0000x reference)
"""Sharded cosine-similarity kNN retrieval kernel for Trainium2 (Bass/Tile).

Problem: one query [D] against keys [N, D]; return actions[top_k indices of
cosine similarity].  N=100000, D=2048, A=7, top_k<=8.

Strategy:
  - Shard keys row-wise across 8 NeuronCores (12544 rows/core, last shard
    zero-padded).  Inputs are downcast to fp16 on the host: halves the HBM
    traffic and lets the DVE run 16-bit ops in 2x mode.  Selection is robust
    to fp16 rounding: top-8 similarity gaps (~1e-3) are ~100x larger than
    the fp16-induced sim error (~1e-5).
  - Per 128-row tile on each core:
      * dots[p]  = sum_d keys[p,d]*q[d]   via VectorE scalar_tensor_tensor
                   (fused multiply + free-dim accumulate, one pass)
      * norms2[p]= sum_d keys[p,d]^2      via ScalarE activation(Square,
                   accum_out=...) for 3 of 4 tiles, via a second VectorE
                   scalar_tensor_tensor for every 4th tile (engine balance:
                   DVE 1.22us + 0.25*1.22us vs ACT 0.75*2.0us per tile).
  - Host: sims = dots / max(|k| * |q|, eps), global top-k over 100k scalars,
    gather actions rows (the standard "reduce M*k candidates" step).
"""

import sys

for _p in ("/opt/trn_rl_repo", "/opt/trn_rl_repo/concourse"):
    if _p not in sys.path:
        sys.path.insert(0, _p)

import numpy as np

import concourse.bacc as bacc
from concourse import mybir
from concourse.bass_utils import run_bass_kernel_spmd
from concourse.tile import TileContext

N, D, A = 100000, 2048, 7
EPS = 1e-8
N_CORES = 8
P = 128
ROWS_PER_CORE = 12544            # 98 tiles of 128 rows; 8*12544 = 100352 >= N
TILES = ROWS_PER_CORE // P       # 98
DMA_CHUNK = 2                    # row-tiles per dma_start
DVE_SQ_MOD = 0                   # 0: all squares on ScalarE (DVE is the
                                 # bottleneck at 2.26us/tile; ACT 2.17us)
USE_FP16 = True

_CACHE = {}


def _build_bass(repeats: int = 1, fp16: bool = USE_FP16,
                dve_sq_mod: int = DVE_SQ_MOD, dma_chunk: int = DMA_CHUNK):
    """Build the per-core Bass program.

    repeats>1 wraps the streaming loop in a hardware For loop that re-reads
    the same DRAM shard; used only for wall-clock HW timing (slope over
    repeats cancels host/axon dispatch overhead)."""
    nc = bacc.Bacc(
        "TRN2",
        target_bir_lowering=False,
        debug=False,
        enable_asserts=False,
        num_devices=N_CORES,
    )
    f32 = mybir.dt.float32
    kdt = mybir.dt.float16 if fp16 else f32
    keys_d = nc.dram_tensor(
        "keys", [ROWS_PER_CORE, D], kdt, kind="ExternalInput"
    ).ap()
    qb_d = nc.dram_tensor("qb", [P, D], kdt, kind="ExternalInput").ap()
    dots_d = nc.dram_tensor(
        "dots", [P, TILES], f32, kind="ExternalOutput"
    ).ap()
    norms2_d = nc.dram_tensor(
        "norms2", [P, TILES], f32, kind="ExternalOutput"
    ).ap()

    # keys viewed as [p, t, d]: row t*128+p  ->  partition p, tile t
    keys_r = keys_d.rearrange("(t p) d -> p t d", p=P)

    with TileContext(nc) as tc:
        with tc.tile_pool(name="kpool", bufs=4) as kpool, \
             tc.tile_pool(name="spool", bufs=2) as spool, \
             tc.tile_pool(name="cpool", bufs=1) as cpool:
            qb_t = cpool.tile([P, D], kdt)
            nc.sync.dma_start(out=qb_t, in_=qb_d)
            dots_t = cpool.tile([P, TILES], f32)
            norms_t = cpool.tile([P, TILES], f32)

            def body():
                for c in range(TILES // dma_chunk):
                    kt = kpool.tile([P, dma_chunk, D], kdt, tag="keys",
                                    name="kt")
                    nc.sync.dma_start(
                        out=kt,
                        in_=keys_r[:, c * dma_chunk:(c + 1) * dma_chunk, :],
                    )
                    for j in range(dma_chunk):
                        t = c * dma_chunk + j
                        prod = spool.tile([P, D], kdt, tag="prod", name="prod")
                        if fp16 and t % 33 == 16:
                            # engine balance: DVE's fused dot (1x, 2.26us)
                            # is the kernel bottleneck; for 3 of 98 tiles
                            # do the multiply at 2x on DVE and let ScalarE
                            # (which has slack) do the reduction.
                            nc.vector.tensor_mul(prod, kt[:, j, :], qb_t)
                            cout = spool.tile([P, D], kdt, tag="cout",
                                              name="cout")
                            nc.scalar.activation(
                                cout,
                                prod,
                                mybir.ActivationFunctionType.Copy,
                                accum_out=dots_t[:, t:t + 1],
                            )
                        else:
                            nc.vector.scalar_tensor_tensor(
                                out=prod,
                                in0=kt[:, j, :],
                                scalar=1.0,
                                in1=qb_t,
                                op0=mybir.AluOpType.bypass,
                                op1=mybir.AluOpType.mult,
                                accum_out=dots_t[:, t:t + 1],
                            )
                        sq = spool.tile([P, D], kdt, tag="sq", name="sq")
                        if dve_sq_mod and t % dve_sq_mod == 0:
                            nc.vector.scalar_tensor_tensor(
                                out=sq,
                                in0=kt[:, j, :],
                                scalar=1.0,
                                in1=kt[:, j, :],
                                op0=mybir.AluOpType.bypass,
                                op1=mybir.AluOpType.mult,
                                accum_out=norms_t[:, t:t + 1],
                            )
                        else:
                            nc.scalar.activation(
                                sq,
                                kt[:, j, :],
                                mybir.ActivationFunctionType.Square,
                                accum_out=norms_t[:, t:t + 1],
                            )

            if repeats == 1:
                body()
            else:
                with tc.For_i(0, repeats, 1):
                    body()

            nc.sync.dma_start(out=dots_d, in_=dots_t)
            nc.sync.dma_start(out=norms2_d, in_=norms_t)
    nc.compile()
    return nc


def _get_nc(repeats: int = 1, **kw):
    key = ("nc", repeats, tuple(sorted(kw.items())))
    if key not in _CACHE:
        _CACHE[key] = _build_bass(repeats, **kw)
    return _CACHE[key]


def _make_in_maps(keys: np.ndarray, query: np.ndarray,
                  fp16: bool = USE_FP16):
    dt = np.float16 if fp16 else np.float32
    qb = np.ascontiguousarray(
        np.broadcast_to(query.astype(dt), (P, D))
    )
    in_maps = []
    for i in range(N_CORES):
        lo, hi = i * ROWS_PER_CORE, (i + 1) * ROWS_PER_CORE
        if hi <= N:
            shard = np.ascontiguousarray(keys[lo:hi], dtype=dt)
        else:
            shard = np.zeros((ROWS_PER_CORE, D), dtype=dt)
            shard[: N - lo] = keys[lo:N].astype(dt)
        in_maps.append({"keys": shard, "qb": qb})
    return in_maps


def _run_device(keys: np.ndarray, query: np.ndarray, trace: bool = False):
    """Run the SPMD kernel; returns (dots[100352], norms2[100352], results)."""
    nc = _get_nc()
    in_maps = _make_in_maps(keys, query)
    res = run_bass_kernel_spmd(
        nc, in_maps, core_ids=list(range(N_CORES)), trace=trace
    )
    dots = np.empty(N_CORES * ROWS_PER_CORE, np.float32)
    norms2 = np.empty(N_CORES * ROWS_PER_CORE, np.float32)
    for i, out in enumerate(res.results):
        # out["dots"][p, t] is row t*128+p of shard i
        base = i * ROWS_PER_CORE
        dots[base:base + ROWS_PER_CORE] = out["dots"].T.reshape(-1)
        norms2[base:base + ROWS_PER_CORE] = out["norms2"].T.reshape(-1)
    return dots, norms2, res


def kernel(**inputs) -> np.ndarray:
    query = np.asarray(inputs["query_key"], dtype=np.float32)
    keys = np.asarray(inputs["keys"], dtype=np.float32)
    actions = np.asarray(inputs["actions"])
    top_k = int(inputs["top_k"])
    if top_k <= 0:
        return actions[:0]
    top_k = min(top_k, keys.shape[0])

    dots, norms2, _ = _run_device(keys, query)
    dots = dots[:N]
    norms2 = norms2[:N]

    q16 = query.astype(np.float16).astype(np.float32)
    q_norm = np.float32(np.linalg.norm(q16))
    denom = np.maximum(np.sqrt(norms2) * q_norm, np.float32(EPS))
    sims = dots / denom

    # top_k, ties resolved to the lower index (jax.lax.top_k semantics)
    cand = np.argpartition(-sims, top_k - 1)[:top_k]
    order = np.lexsort((cand, -sims[cand]))
    idx = cand[order]
    return actions[idx]

